# revision 1
# baseline (speedup 1.0000x reference)
"""Trainium2 Bass kernel for nn_GeneralizedKernelScore (loss_fn).

Math per sample n (M=8 population members, D=12288 features):
    beta      = 2.0 - 1.9*t/999                      (linear schedule from t)
    conf[n]   = mean_j    exp(-beta*||x_j - y_j||^2 / D)
    inter[n]  = mean_{j!=j'} exp(-beta*||x_j - x_j'||^2 / D)
    im[n]     = inter/2
    score[n]  = im - conf

Strategy (data-parallel over batch, 4 samples per core on 8 cores):
Each core owns Z = [X; Y] (64 rows x 12288) in fp8-e4m3, pre-transposed
on the host to feature-major [128, 96*64] so the contraction dim lands
on SBUF partitions.  All distances come from the Gram matrix G = Z Z^T.
Feature chunks are processed two at a time as column-group-tiled
matmul pairs that stream CONCURRENTLY on distinct halves of the PE
array (tile positions (0,0)/(0,64) auto-derived from the PSUM slices):
    P[0:64, :]   += chunk_2j   Gram contribution   (cols 0-63)
    P[64:128, :] += chunk_2j+1 Gram contribution   (cols 64-127)
giving ~32-53 ns per chunk pair with the 64-col weight loads hidden
behind the opposite group's stream.  A short warm-up spin of junk
matmuls plus filler matmuls in DMA gaps keep the PE busy so the HAM
clock gate reaches 2.4 GHz mid-stream.

Epilogue (few cross-engine hops):
  DVE   : xn2 = diag(G) via masked reduce; one fused tensor_scalar
          builds the norm-routing rhs and the fold weights; one
          combined mask (same-sample block + x.y diag, disjoint) +
          grouped reduce compacts the -2G terms, with the x.y term
          landing in the f = p%8 slot
  PE    : three matmuls accumulate pt[:,0:8] = D*d2 args; the diag
          slot becomes the confinement arg, the [128->32] fold of the
          split Gram halves rides the contraction
  DVE   : copy the diag slot (conf arg) into pt col 8 before the exp
  ACT   : one Exp over [32,9] with per-partition scale -beta/D
          (host-computed from t)
  PE    : per-sample sums of the 9 exp columns (selection matmul)
  DVE+DMA: copy [4,9] out (few-line outputs quiesce fastest)
  Host  : folds the 9 columns per sample with the constant affine.

DMA: input split in 4 chunks (small first chunk for an early start)
issued alternately on the two HWDGE queues (SP + Activation) so the
rings drain in parallel; constants ride a 5th transfer.
"""

from contextlib import ExitStack

import numpy as np
import ml_dtypes

import concourse.bass as bass
from concourse.bass_types import AP
import concourse.mybir as mybir
import concourse.tile as tile
from concourse import bacc
from concourse.bass_utils import run_bass_kernel_spmd

# problem shape (hardcoded per spec)
N, M, D = 32, 8, 12288
NUM_TIMESTEPS = 1000
BETA_START, BETA_END = 2.0, 0.1
LAMBDA_VAL = 1.0

NCORES = 8
NS = N // NCORES          # 4 samples per core
R = 2 * NS * M            # 64 Z-rows per core (32 x-rows then 32 y-rows)
NCH = D // 128            # 96 contraction chunks of the feature dim
FREE = NCH * R            # 6144 free columns of Z^T
# input DMA chunk widths (bytes per partition line); must sum to FREE
# and stay multiples of 128 (one ldw-pair)
CHUNKS = [256, 1024, 2816, 2048]
N_WARM = 9                # PE warm-up matmuls (N=256 each)
FILLERS = [2, 2, 0, 0]    # junk matmuls after each chunk's pairs: keep the
                          # PE busy through DMA gaps so HAM stays warming
DIAG_AP = False           # stride-129 diag AP (rejected by birverifier)

# const tensor column layout
_M2C, _I64, _MK8, _W3, _W2, _ON8, _MD, _BV, _P4 = (
    0, 64, 128, 136, 168, 200, 208, 216, 217,
)
CONW = 221

F32 = mybir.dt.float32
FP8 = mybir.dt.float8e4
NP_FP8 = ml_dtypes.float8_e4m3


def _build_consts():
    k = np.arange(128)[:, None]
    km = k % 64
    c = np.arange(64)[None, :]
    xrow = km < 32
    # combined -2 mask: same-sample x-x block (incl diag) + x.y diag;
    # disjoint regions, both land compatibly under the g=8 grouped sum
    m2c = np.where(
        (xrow & (c // 8 == km // 8) & (c < 32)) | (xrow & (c == km + 32)),
        -2.0, 0.0,
    )
    i64 = (c == km).astype(np.float32)  # diag mask per half
    f8 = np.arange(8)[None, :]
    mk8 = (k % 8 == f8).astype(np.float32)       # norm routing by j = k%8
    m32 = np.arange(32)[None, :]
    w3 = (xrow & (km == m32)).astype(np.float32)  # fold [128]->[32], x-rows
    # W2 = A (same-sample x-rows) + B (own y-row) + C (own x-row);
    # arithmetic sum: A and C overlap on the own row, weight 2 there
    w2 = (
        (xrow & (km // 8 == m32 // 8)).astype(np.float32)
        + (km == 32 + m32).astype(np.float32)
        + (km == m32).astype(np.float32)
    )
    on8 = np.ones((128, 8), dtype=np.float32)
    md = (xrow & (k % 8 == f8)).astype(np.float32)[: 128]  # diag-slot mask
    bv = np.zeros((128, 1), dtype=np.float32)  # filled per-core with -beta/D
    p4 = ((k < 32) & (k // 8 == np.arange(4)[None, :])).astype(np.float32)
    con = np.concatenate(
        [m2c, i64, mk8, w3, w2, on8, md, bv, p4], axis=1
    ).astype(np.float32)
    assert con.shape == (128, CONW)
    return con


def _build_program():
    nc = bacc.Bacc("TRN2", target_bir_lowering=False)
    zt = nc.dram_tensor("zt", [128, FREE], FP8, kind="ExternalInput")
    con_d = nc.dram_tensor("con", [128, CONW], F32, kind="ExternalInput")
    res_d = nc.dram_tensor("res", [NS, 9], F32, kind="ExternalOutput")

    mult = mybir.AluOpType.mult
    EXP = mybir.ActivationFunctionType.Exp

    with ExitStack() as ctx:
        tc = ctx.enter_context(tile.TileContext(nc))
        small = ctx.enter_context(tc.tile_pool(name="small", bufs=1))
        zbf_p = ctx.enter_context(tc.tile_pool(name="zbf", bufs=len(CHUNKS)))
        psum = ctx.enter_context(tc.tile_pool(name="psum", bufs=1, space="PSUM"))

        # --- PE warm-up spin: open the HAM clock gate early -----------
        wt = small.tile([128, 256], FP8, tag="wt")
        nc.vector.memset(wt, 0.0)
        wp = psum.tile([128, 256], F32, tag="wp")
        for _ in range(N_WARM):
            nc.tensor.matmul(
                wp, lhsT=wt[:, 0:128], rhs=wt, start=True, stop=True,
                skip_group_check=True,
            )

        # --- input + const DMAs, alternating the two HWDGE queues -----
        zbf = []
        off = 0
        for i, w in enumerate(CHUNKS):
            zc = zbf_p.tile([128, w], FP8, tag="zbf")
            eng = nc.sync if i % 2 == 0 else nc.scalar
            eng.dma_start(out=zc, in_=zt[:, off : off + w])
            zbf.append(zc)
            off += w
        con = small.tile([128, CONW], F32, tag="con")
        nc.sync.dma_start(out=con, in_=con_d[:])

        # preload the Exp LUT while DMAs run
        warm = small.tile([1, 1], F32, tag="warm")
        nc.vector.memset(warm, 0.0)
        nc.scalar.activation(out=warm, in_=warm, func=EXP)

        # --- Gram: one [128,128] matmul per chunk pair ----------------
        G = psum.tile([128, 64], F32, tag="G")
        npair = NCH // 2
        p = 0
        for i, cw in enumerate(CHUNKS):
            for j in range(cw // 128):
                a = zbf[i][:, j * 128 : j * 128 + 64]
                b = zbf[i][:, j * 128 + 64 : (j + 1) * 128]
                nc.tensor.matmul(
                    G[0:64, :], lhsT=a, rhs=a,
                    start=(p == 0), stop=(p == npair - 1),
                    skip_group_check=True,
                )
                nc.tensor.matmul(
                    G[64:128, :], lhsT=b, rhs=b,
                    start=(p == 0), stop=(p == npair - 1),
                    skip_group_check=True,
                )
                p += 1
            for _ in range(FILLERS[i]):
                nc.tensor.matmul(
                    wp, lhsT=wt[:, 0:128], rhs=wt, start=True, stop=True,
                    skip_group_check=True,
                )
        assert p == npair

        # --- epilogue ---------------------------------------------------
        # [V] xn2 = diag(G): split norms (even-chunk half on rows 0-63,
        # odd on 64-127)
        xn2 = small.tile([128, 1], F32, tag="xn2")
        if DIAG_AP:
            gdiag = AP(tensor=G.tensor, offset=G.offset, ap=[[129, 128], [1, 1]])
            nc.vector.tensor_copy(out=xn2, in_=gdiag)
        else:
            s128 = small.tile([128, 64], F32, tag="s128")
            nc.vector.tensor_tensor(
                out=s128, in0=G, in1=con[:, _I64 : _I64 + 64], op=mult
            )
            nc.vector.reduce_sum(out=xn2, in_=s128, axis=mybir.AxisListType.X)
        # rw = [mask8 | W3] . xn2 : rhs8 = rw[:,0:8], xnw = rw[:,8:40]
        rw = small.tile([128, 40], F32, tag="rw")
        nc.vector.tensor_scalar(
            out=rw, in0=con[:, _MK8 : _MK8 + 40], scalar1=xn2, scalar2=None,
            op0=mult,
        )
        gm = small.tile([128, 64], F32, tag="gm")
        nc.vector.tensor_tensor(
            out=gm, in0=G, in1=con[:, _M2C : _M2C + 64], op=mult
        )
        cmc = small.tile([128, 8], F32, tag="cmc")
        nc.vector.reduce_sum(
            out=cmc,
            in_=gm.rearrange("p (g f) -> p f g", g=8),
            axis=mybir.AxisListType.X,
        )

        # [T] pt[:,0:8] = norm-spread + row-norm bcast + (-2G, folded)
        pt = psum.tile([32, 9], F32, tag="pt")
        nc.tensor.matmul(
            pt[:, 0:8], lhsT=con[:, _W2 : _W2 + 32], rhs=rw[:, 0:8],
            start=True, stop=False,
        )
        nc.tensor.matmul(
            pt[:, 0:8], lhsT=rw[:, 8:40], rhs=con[:, _ON8 : _ON8 + 8],
            start=False, stop=False,
        )
        nc.tensor.matmul(
            pt[:, 0:8], lhsT=con[:, _W3 : _W3 + 32], rhs=cmc,
            start=False, stop=True,
        )

        # [V] copy the confinement arg (diag slot) into pt col 8
        md = small.tile([32, 8], F32, tag="md")
        nc.vector.tensor_tensor(
            out=md, in0=pt[:, 0:8], in1=con[0:32, _MD : _MD + 8], op=mult
        )
        nc.vector.reduce_sum(
            out=pt[:, 8:9], in_=md, axis=mybir.AxisListType.X
        )

        # [S] one exp over [32,9]: col 8 = conf, cols 0-7 pairs (diag
        # slot also conf)
        w = small.tile([32, 9], F32, tag="w")
        nc.scalar.activation(
            out=w, in_=pt, func=EXP, scale=con[0:32, _BV : _BV + 1]
        )

        # [T] per-sample sums over the 8 population rows (all 9 cols;
        # host folds the columns)
        pc = psum.tile([NS, 9], F32, tag="pc")
        nc.tensor.matmul(
            pc, lhsT=con[0:32, _P4 : _P4 + 4], rhs=w, start=True, stop=True
        )

        # [V] -> DMA out
        fin = small.tile([NS, 9], F32, tag="fin")
        nc.vector.tensor_copy(out=fin, in_=pc)
        nc.sync.dma_start(out=res_d[:], in_=fin)

    nc.compile()
    return nc


_PROG = None
_CONSTS = None


def _get_prog():
    global _PROG
    if _PROG is None:
        _PROG = _build_program()
    return _PROG


def _make_in_maps(x, y, t):
    global _CONSTS
    if _CONSTS is None:
        _CONSTS = _build_consts()
    beta = BETA_START + (BETA_END - BETA_START) * (
        t.astype(np.float64) / (NUM_TIMESTEPS - 1)
    )
    in_maps = []
    for c in range(NCORES):
        xc = x[c * NS : (c + 1) * NS].reshape(NS * M, D)
        yc = y[c * NS : (c + 1) * NS].reshape(NS * M, D)
        z = np.concatenate([xc, yc], axis=0)  # [64, D]
        # feature-major: zt[p, k*64 + r] = z[r, k*128 + p]
        zt = np.ascontiguousarray(
            z.reshape(R, NCH, 128).transpose(2, 1, 0).reshape(128, FREE)
        ).astype(NP_FP8)
        con = _CONSTS.copy()
        bcore = np.repeat(beta[c * NS : (c + 1) * NS], M)  # [32]
        con[0:32, _BV] = (-bcore / D).astype(np.float32)
        in_maps.append({"zt": zt, "con": con})
    return in_maps


def _run(x, y, t, trace=False, **spmd_kwargs):
    x = np.asarray(x, dtype=np.float32)
    y = np.asarray(y, dtype=np.float32)
    t = np.asarray(t, dtype=np.int32)
    nc = _get_prog()
    in_maps = _make_in_maps(x, y, t)
    br = run_bass_kernel_spmd(
        nc, in_maps, list(range(NCORES)), trace=trace, **spmd_kwargs
    )
    S = np.concatenate(
        [np.asarray(r["res"], dtype=np.float32) for r in br.results], axis=0
    )  # [32, 9]: per-sample sums of the 9 exp columns; col 8 = conf
    conf_sum = S[:, 8]
    pairs = S.sum(axis=1) - 2.0 * conf_sum
    conf = conf_sum / M
    inter = pairs / (M * (M - 1))
    im = (LAMBDA_VAL / 2.0) * inter
    score = im - conf
    outs = tuple(
        np.ascontiguousarray(v, dtype=np.float32)
        for v in (score, conf, inter, im)
    )
    return outs, br


def kernel(x, y, t):
    """(score, confinement, interaction, interaction_mult), each [32] f32."""
    outs, _ = _run(x, y, t)
    return outs



# revision 7
# speedup vs baseline: 1.0353x; 1.0353x over previous
"""Trainium2 Bass kernel for nn_GeneralizedKernelScore (loss_fn).

Math per sample n (M=8 population members, D=12288 features):
    beta      = 2.0 - 1.9*t/999                      (linear schedule from t)
    conf[n]   = mean_j    exp(-beta*||x_j - y_j||^2 / D)
    inter[n]  = mean_{j!=j'} exp(-beta*||x_j - x_j'||^2 / D)
    im[n]     = inter/2
    score[n]  = im - conf

Strategy (data-parallel over batch, 4 samples per core on 8 cores):
Each core owns Z = [X; Y] (64 rows x 12288) in fp8-e4m3, pre-transposed
on the host to feature-major [128, 96*64] so the contraction dim lands
on SBUF partitions.  All distances come from the Gram matrix G = Z Z^T.

STREAM_MODE selects the Gram accumulation:
  "dr"    — fp8 DoubleRow matmuls (K=256 per instruction, 48 total) into
            a single unsplit [64,64] PSUM Gram.  DoubleRow is only valid
            at tile position (0,0) / psum base 0, so there is no
            LDWEIGHTS/MATMUL column-group ping-pong.
  "pairs" — two normal matmuls (K=128) per chunk pair on distinct PE
            column groups (psum halves fold later), 96 instructions with
            weight loads hidden behind the opposite group's stream.

Input streams over both HWDGE queues (sync + scalar) in 3 chunks per
queue; the (small) mask constants ride at the tail of the sync queue.

Epilogue: DVE tensor_tensor_reduce extracts xn2 = diag(G) in one op;
tensor_scalar spreads it through [mk8 | w3] routing; masked -2G
compaction (m2c mult + grouped reduce); three f32 PE matmuls assemble
pt[32,8] = D*d2 args (col j==f is the confinement arg); one Exp over
[32,8] with per-partition scale -beta/D; DMA [32,8] out; host folds.
"""

from contextlib import ExitStack

import numpy as np
import ml_dtypes

import concourse.bass as bass
import concourse.mybir as mybir
import concourse.tile as tile
from concourse import bacc
from concourse.bass_utils import run_bass_kernel_spmd

# problem shape (hardcoded per spec)
N, M, D = 32, 8, 12288
NUM_TIMESTEPS = 1000
BETA_START, BETA_END = 2.0, 0.1
LAMBDA_VAL = 1.0

NCORES = 8
NS = N // NCORES          # 4 samples per core
R = 2 * NS * M            # 64 Z-rows per core (32 x-rows then 32 y-rows)

DK = 12288                # feature subsample (<= D, multiple of 256)
NCH = DK // 128           # contraction chunks of the feature dim
NPAIR = NCH // 2          # chunk pairs
FREE = NCH * R            # free columns of Z^T

STREAM_MODE = "dr"        # "dr" (DoubleRow, unsplit G) | "pairs" (split G)
HALVES = STREAM_MODE == "pairs"
GP = 128 if HALVES else 64   # partition rows carrying Gram data

# chunk widths (columns), alternating sync/scalar queues
CHUNKS = [512, 512, 1024, 1024, 1536, 1536]
assert sum(CHUNKS) == FREE and all(c % 128 == 0 for c in CHUNKS)

N_WARM = 6                # PE warm-up matmuls to open the HAM clock gate
POST_SPIN = 0             # junk matmuls after the output DMA (clock probe)

# conf (f32) column layout
_MK8, _W3, _W2, _ON8, _BV = 0, 8, 40, 72, 80
CONF_W = 81
# conb (fp8) column layout
_M2C, _I64 = 0, 64
CONB_W = 128

F32 = mybir.dt.float32
BF16 = mybir.dt.bfloat16
FP8 = mybir.dt.float8e4
NP_FP8 = ml_dtypes.float8_e4m3


def _build_consts():
    k = np.arange(GP)[:, None]
    km = k % 64                      # z-row of this partition
    xrow = km < 32
    c = np.arange(64)[None, :]
    # -2 mask: same-sample x-x block (incl diag) + own x.y diag
    m2c = np.where(
        (xrow & (c // 8 == km // 8) & (c < 32)) | (xrow & (c == km + 32)),
        -2.0, 0.0,
    )
    i64 = (c == km).astype(np.float32)            # diag mask
    f8 = np.arange(8)[None, :]
    mk8 = (km % 8 == f8).astype(np.float32)       # norm routing by j = r%8
    m32 = np.arange(32)[None, :]
    w3 = (xrow & (km == m32)).astype(np.float32)  # fold [GP]->[32], x-rows
    # W2 = A (same-sample x-rows) + B (own y-row) + C (own x-row)
    w2 = (
        (xrow & (km // 8 == m32 // 8)).astype(np.float32)
        + (km == 32 + m32).astype(np.float32)
        + (km == m32).astype(np.float32)
    )
    on8 = np.ones((GP, 8), dtype=np.float32)
    bv = np.zeros((GP, 1), dtype=np.float32)      # per-core -beta/DK
    conf = np.concatenate([mk8, w3, w2, on8, bv], axis=1).astype(np.float32)
    conb = np.concatenate([m2c, i64], axis=1).astype(NP_FP8)
    assert conf.shape == (GP, CONF_W) and conb.shape == (GP, CONB_W)
    return conf, conb


def _build_program():
    nc = bacc.Bacc("TRN2", target_bir_lowering=False)
    zt = nc.dram_tensor("zt", [128, FREE], FP8, kind="ExternalInput")
    conf_d = nc.dram_tensor("conf", [GP, CONF_W], F32, kind="ExternalInput")
    conb_d = nc.dram_tensor("conb", [GP, CONB_W], FP8, kind="ExternalInput")
    res_d = nc.dram_tensor("res", [32, 8], F32, kind="ExternalOutput")

    mult = mybir.AluOpType.mult
    add = mybir.AluOpType.add
    EXP = mybir.ActivationFunctionType.Exp
    DR = mybir.MatmulPerfMode.DoubleRow

    with ExitStack() as ctx:
        tc = ctx.enter_context(tile.TileContext(nc))
        small = ctx.enter_context(tc.tile_pool(name="small", bufs=1))
        zpool = ctx.enter_context(tc.tile_pool(name="z", bufs=1))
        psum = ctx.enter_context(tc.tile_pool(name="psum", bufs=1, space="PSUM"))

        # --- input + const DMAs, alternating the two HWDGE queues -----
        zbf = []
        off = 0
        for i, cw in enumerate(CHUNKS):
            zc = zpool.tile([128, cw // 64, 64], FP8, tag=f"z{i}")
            eng = nc.sync if i % 2 == 0 else nc.scalar
            eng.dma_start(out=zc, in_=zt[:, off : off + cw])
            zbf.append(zc)
            off += cw
        conb = small.tile([GP, CONB_W], FP8, tag="conb")
        conf = small.tile([GP, CONF_W], F32, tag="conf")
        nc.sync.dma_start(out=conb, in_=conb_d[:])
        nc.sync.dma_start(out=conf, in_=conf_d[:])

        # --- PE warm-up spin: open the HAM clock gate early -----------
        wt = small.tile([128, 2, 64], FP8, tag="wt")
        nc.vector.memset(wt, 0.0)
        wp = psum.tile([64, 64], F32, tag="wp")

        def spin(i):
            nc.tensor.matmul(
                wp, lhsT=wt, rhs=wt, start=True, stop=True, perf_mode=DR,
                tile_position=(0, 0), skip_group_check=True,
            )

        for i in range(N_WARM):
            spin(i)

        # preload the Exp LUT while DMAs run
        warm = small.tile([1, 1], F32, tag="warm")
        nc.scalar.activation(out=warm, in_=wp[0:1, 0:1], func=EXP)

        # --- Gram accumulation ---------------------------------------
        G = psum.tile([GP, 64], F32, tag="G")
        p = 0
        for i, cw in enumerate(CHUNKS):
            for j in range(cw // 128):
                if STREAM_MODE == "dr":
                    seg = zbf[i][:, 2 * j : 2 * j + 2, :]
                    nc.tensor.matmul(
                        G, lhsT=seg, rhs=seg,
                        start=(p == 0), stop=(p == NPAIR - 1),
                        perf_mode=DR, tile_position=(0, 0),
                        skip_group_check=True,
                    )
                else:
                    a = zbf[i][:, 2 * j, :]
                    b = zbf[i][:, 2 * j + 1, :]
                    nc.tensor.matmul(
                        G[0:64, :], lhsT=a, rhs=a,
                        start=(p == 0), stop=(p == NPAIR - 1),
                        skip_group_check=True,
                    )
                    nc.tensor.matmul(
                        G[64:128, :], lhsT=b, rhs=b,
                        start=(p == 0), stop=(p == NPAIR - 1),
                        skip_group_check=True,
                    )
                p += 1
        assert p == NPAIR

        # --- epilogue: PSUM-reading ops on DVE (gpsimd cannot touch
        # PSUM; tensor_tensor_reduce faults TRN2 hw), rw spread on GPS
        junk = small.tile([GP, 64], F32, tag="junk")
        xn2 = small.tile([GP, 1], F32, tag="xn2")
        nc.vector.tensor_tensor(
            out=junk, in0=G, in1=conb[:, _I64 : _I64 + 64], op=mult
        )
        nc.vector.reduce_sum(out=xn2, in_=junk, axis=mybir.AxisListType.X)
        rw = small.tile([GP, 40], F32, tag="rw")
        nc.gpsimd.tensor_scalar(
            out=rw, in0=conf[:, _MK8 : _MK8 + 40], scalar1=xn2, scalar2=None,
            op0=mult,
        )
        gm = small.tile([GP, 64], F32, tag="gm")
        nc.vector.tensor_tensor(
            out=gm, in0=G, in1=conb[:, _M2C : _M2C + 64], op=mult
        )
        cmc = small.tile([GP, 8], F32, tag="cmc")
        nc.vector.reduce_sum(
            out=cmc,
            in_=gm.rearrange("p (g f) -> p f g", g=8),
            axis=mybir.AxisListType.X,
        )

        # [PE] pt[:,0:8] = norm-spread + row-norm bcast + (-2G, folded)
        pt = psum.tile([32, 8], F32, tag="pt")
        nc.tensor.matmul(
            pt, lhsT=conf[:, _W2 : _W2 + 32], rhs=rw[:, 0:8],
            start=True, stop=False,
        )
        nc.tensor.matmul(
            pt, lhsT=rw[:, 8:40], rhs=conf[:, _ON8 : _ON8 + 8],
            start=False, stop=False,
        )
        nc.tensor.matmul(
            pt, lhsT=conf[:, _W3 : _W3 + 32], rhs=cmc,
            start=False, stop=True,
        )

        # [ACT] exp over [32,8] with per-partition scale -beta/DK
        w = small.tile([32, 8], F32, tag="w")
        nc.scalar.activation(
            out=w, in_=pt, func=EXP, scale=conf[0:32, _BV : _BV + 1]
        )

        # -> DMA out
        nc.sync.dma_start(out=res_d[:], in_=w)

        # postamble clock probe: keep the PE sequencer hot at the end
        for i in range(POST_SPIN):
            spin(i)

    nc.compile()
    return nc


_PROG = None
_CONSTS = None


def _get_prog():
    global _PROG
    if _PROG is None:
        _PROG = _build_program()
    return _PROG


def _make_in_maps(x, y, t):
    global _CONSTS
    if _CONSTS is None:
        _CONSTS = _build_consts()
    conf0, conb = _CONSTS
    beta = BETA_START + (BETA_END - BETA_START) * (
        t.astype(np.float64) / (NUM_TIMESTEPS - 1)
    )
    in_maps = []
    for c in range(NCORES):
        xc = x[c * NS : (c + 1) * NS].reshape(NS * M, D)
        yc = y[c * NS : (c + 1) * NS].reshape(NS * M, D)
        z = np.concatenate([xc, yc], axis=0)[:, :DK]  # [64, DK]
        # feature-major: zt[p, k*64 + r] = z[r, k*128 + p]
        zt = np.ascontiguousarray(
            z.reshape(R, NCH, 128).transpose(2, 1, 0).reshape(128, FREE)
        ).astype(NP_FP8)
        conf = conf0.copy()
        bcore = np.repeat(beta[c * NS : (c + 1) * NS], M)  # [32]
        conf[0:32, _BV] = (-bcore / DK).astype(np.float32)
        in_maps.append({"zt": zt, "conf": conf, "conb": conb})
    return in_maps


def _fold(res):
    """res [32, 8] per core -> (conf_sum, pair_sum) per sample [4]."""
    w = res.reshape(NS, M, M)                      # [s, j, f]
    conf_sum = np.einsum("sjj->s", w)
    total = w.sum(axis=(1, 2))
    return conf_sum, total - conf_sum


def _run(x, y, t, trace=False, **spmd_kwargs):
    x = np.asarray(x, dtype=np.float32)
    y = np.asarray(y, dtype=np.float32)
    t = np.asarray(t, dtype=np.int32)
    nc = _get_prog()
    in_maps = _make_in_maps(x, y, t)
    br = run_bass_kernel_spmd(
        nc, in_maps, list(range(NCORES)), trace=trace, **spmd_kwargs
    )
    confs, pairs = [], []
    for r in br.results:
        cs, ps = _fold(np.asarray(r["res"], dtype=np.float32))
        confs.append(cs)
        pairs.append(ps)
    conf_sum = np.concatenate(confs)
    pair_sum = np.concatenate(pairs)
    conf = conf_sum / M
    inter = pair_sum / (M * (M - 1))
    im = (LAMBDA_VAL / 2.0) * inter
    score = im - conf
    outs = tuple(
        np.ascontiguousarray(v, dtype=np.float32)
        for v in (score, conf, inter, im)
    )
    return outs, br


def kernel(x, y, t):
    """(score, confinement, interaction, interaction_mult), each [32] f32."""
    outs, _ = _run(x, y, t)
    return outs


# revision 14
# speedup vs baseline: 1.0927x; 1.0554x over previous
"""Trainium2 Bass kernel for nn_GeneralizedKernelScore (loss_fn).

Math per sample n (M=8 population members, D=12288 features):
    beta      = 2.0 - 1.9*t/999                      (linear schedule from t)
    conf[n]   = mean_j    exp(-beta*||x_j - y_j||^2 / D)
    inter[n]  = mean_{j!=j'} exp(-beta*||x_j - x_j'||^2 / D)
    im[n]     = inter/2
    score[n]  = im - conf

Strategy (data-parallel over batch, 4 samples per core on 8 cores):
Each core owns Z = [X; Y] (64 rows x 12288) in fp8-e4m3, pre-transposed
on the host to feature-major [128, 96*64] so the contraction dim lands
on SBUF partitions.  All distances come from the Gram matrix G = Z Z^T.

STREAM_MODE selects the Gram accumulation:
  "dr"    — fp8 DoubleRow matmuls (K=256 per instruction, 48 total) into
            a single unsplit [64,64] PSUM Gram.  DoubleRow is only valid
            at tile position (0,0) / psum base 0, so there is no
            LDWEIGHTS/MATMUL column-group ping-pong.
  "pairs" — two normal matmuls (K=128) per chunk pair on distinct PE
            column groups (psum halves fold later), 96 instructions with
            weight loads hidden behind the opposite group's stream.

Input streams over both HWDGE queues (sync + scalar) in 3 chunks per
queue; the (small) mask constants ride at the tail of the sync queue.

Epilogue: DVE tensor_tensor_reduce extracts xn2 = diag(G) in one op;
tensor_scalar spreads it through [mk8 | w3] routing; masked -2G
compaction (m2c mult + grouped reduce); three f32 PE matmuls assemble
pt[32,8] = D*d2 args (col j==f is the confinement arg); one Exp over
[32,8] with per-partition scale -beta/D; DMA [32,8] out; host folds.
"""

from contextlib import ExitStack

import numpy as np
import ml_dtypes

import concourse.bass as bass
import concourse.mybir as mybir
import concourse.tile as tile
from concourse import bacc
from concourse.bass_utils import run_bass_kernel_spmd

# problem shape (hardcoded per spec)
N, M, D = 32, 8, 12288
NUM_TIMESTEPS = 1000
BETA_START, BETA_END = 2.0, 0.1
LAMBDA_VAL = 1.0

NCORES = 8
NS = N // NCORES          # 4 samples per core
R = 2 * NS * M            # 64 Z-rows per core (32 x-rows then 32 y-rows)

DK = 12288                # feature subsample (<= D, multiple of 256)
NCH = DK // 128           # contraction chunks of the feature dim
NPAIR = NCH // 2          # chunk pairs
FREE = NCH * R            # free columns of Z^T

STREAM_MODE = "dr"        # DoubleRow (K=256/instr), unsplit [64,64] G
assert STREAM_MODE == "dr"   # epilogue assumes the unsplit Gram
GP = 64                      # partition rows carrying Gram data

# chunk widths (columns), alternating sync/scalar queues
CHUNKS = [256, 256, 1280, 1280, 1536, 1536]
assert sum(CHUNKS) == FREE and all(c % 128 == 0 for c in CHUNKS)

N_WARM = 6                # PE warm-up matmuls to open the HAM clock gate
POST_SPIN = 0             # junk matmuls after the output DMA (clock probe)

# conf (f32) column layout
_MK8, _W2, _BV = 0, 8, 40
CONF_W = 41
# conb (fp8) column layout
_M2C, _I64 = 0, 64
CONB_W = 128

F32 = mybir.dt.float32
BF16 = mybir.dt.bfloat16
FP8 = mybir.dt.float8e4
NP_FP8 = ml_dtypes.float8_e4m3


def _build_consts():
    k = np.arange(GP)[:, None]
    km = k % 64                      # z-row of this partition
    xrow = km < 32
    c = np.arange(64)[None, :]
    # -2 mask: same-sample x-x block (incl diag) + own x.y diag
    m2c = np.where(
        (xrow & (c // 8 == km // 8) & (c < 32)) | (xrow & (c == km + 32)),
        -2.0, 0.0,
    )
    i64 = (c == km).astype(np.float32)            # diag mask
    f8 = np.arange(8)[None, :]
    mk8 = (km % 8 == f8).astype(np.float32)       # norm routing by j = r%8
    m32 = np.arange(32)[None, :]
    # W2 = A (same-sample x-rows) + B (own y-row) + C (own x-row); the
    # all-cols own-norm broadcast (baseline mm2) rides the ACT bias
    w2 = (
        (xrow & (km // 8 == m32 // 8)).astype(np.float32)
        + (km == 32 + m32).astype(np.float32)
        + (km == m32).astype(np.float32)
    )
    bv = np.zeros((GP, 1), dtype=np.float32)      # per-core -beta/DK
    conf = np.concatenate([mk8, w2, bv], axis=1).astype(np.float32)
    conb = np.concatenate([m2c, i64], axis=1).astype(NP_FP8)
    assert conf.shape == (GP, CONF_W) and conb.shape == (GP, CONB_W)
    return conf, conb


def _build_program():
    nc = bacc.Bacc("TRN2", target_bir_lowering=False)
    zt = nc.dram_tensor("zt", [128, FREE], FP8, kind="ExternalInput")
    conf_d = nc.dram_tensor("conf", [GP, CONF_W], F32, kind="ExternalInput")
    conb_d = nc.dram_tensor("conb", [GP, CONB_W], FP8, kind="ExternalInput")
    res_d = nc.dram_tensor("res", [32, 8], F32, kind="ExternalOutput")

    mult = mybir.AluOpType.mult
    add = mybir.AluOpType.add
    EXP = mybir.ActivationFunctionType.Exp
    DR = mybir.MatmulPerfMode.DoubleRow

    with ExitStack() as ctx:
        tc = ctx.enter_context(tile.TileContext(nc))
        small = ctx.enter_context(tc.tile_pool(name="small", bufs=1))
        zpool = ctx.enter_context(tc.tile_pool(name="z", bufs=1))
        psum = ctx.enter_context(tc.tile_pool(name="psum", bufs=1, space="PSUM"))

        # --- input + const DMAs, alternating the two HWDGE queues -----
        zbf = []
        off = 0
        for i, cw in enumerate(CHUNKS):
            zc = zpool.tile([128, cw // 64, 64], FP8, tag=f"z{i}")
            eng = nc.sync if i % 2 == 0 else nc.scalar
            eng.dma_start(out=zc, in_=zt[:, off : off + cw])
            zbf.append(zc)
            off += cw
        conb = small.tile([GP, CONB_W], FP8, tag="conb")
        conf = small.tile([GP, CONF_W], F32, tag="conf")
        nc.scalar.dma_start(out=conb, in_=conb_d[:])
        nc.scalar.dma_start(out=conf, in_=conf_d[:])

        # --- PE warm-up spin: open the HAM clock gate early -----------
        wt = small.tile([128, 2, 64], FP8, tag="wt")
        nc.vector.memset(wt, 0.0)
        wp = psum.tile([64, 64], F32, tag="wp")

        def spin(i):
            nc.tensor.matmul(
                wp, lhsT=wt, rhs=wt, start=True, stop=True, perf_mode=DR,
                tile_position=(0, 0), skip_group_check=True,
            )

        for i in range(N_WARM):
            spin(i)

        # preload the Exp LUT while DMAs run
        warm = small.tile([1, 1], F32, tag="warm")
        nc.scalar.activation(out=warm, in_=wp[0:1, 0:1], func=EXP)

        # --- Gram accumulation ---------------------------------------
        G = psum.tile([GP, 64], F32, tag="G")
        p = 0
        for i, cw in enumerate(CHUNKS):
            for j in range(cw // 128):
                if STREAM_MODE == "dr":
                    seg = zbf[i][:, 2 * j : 2 * j + 2, :]
                    nc.tensor.matmul(
                        G, lhsT=seg, rhs=seg,
                        start=(p == 0), stop=(p == NPAIR - 1),
                        perf_mode=DR, tile_position=(0, 0),
                        skip_group_check=True,
                    )
                else:
                    a = zbf[i][:, 2 * j, :]
                    b = zbf[i][:, 2 * j + 1, :]
                    nc.tensor.matmul(
                        G[0:64, :], lhsT=a, rhs=a,
                        start=(p == 0), stop=(p == NPAIR - 1),
                        skip_group_check=True,
                    )
                    nc.tensor.matmul(
                        G[64:128, :], lhsT=b, rhs=b,
                        start=(p == 0), stop=(p == NPAIR - 1),
                        skip_group_check=True,
                    )
                p += 1
        assert p == NPAIR

        # --- epilogue (PSUM readers must be DVE; tensor_tensor_reduce
        # faults TRN2 hw; gpsimd per-op overhead is ~800ns — avoid) ----
        junk = small.tile([GP, 64], F32, tag="junk")
        xn2 = small.tile([GP, 1], F32, tag="xn2")
        nc.vector.tensor_tensor(
            out=junk, in0=G, in1=conb[:, _I64 : _I64 + 64], op=mult
        )
        nc.vector.reduce_sum(out=xn2, in_=junk, axis=mybir.AxisListType.X)
        # bias = -beta/DK * ||x_j||^2 (own-norm term enters via ACT bias)
        bias = small.tile([32, 1], F32, tag="bias")
        nc.vector.tensor_tensor(
            out=bias, in0=xn2[0:32, :], in1=conf[0:32, _BV : _BV + 1],
            op=mult,
        )
        rw = small.tile([GP, 8], F32, tag="rw")
        nc.vector.tensor_scalar(
            out=rw, in0=conf[:, _MK8 : _MK8 + 8], scalar1=xn2, scalar2=None,
            op0=mult,
        )
        # [PE] pt = other-norm spread (runs while DVE compacts -2G)
        pt = psum.tile([32, 8], F32, tag="pt")
        nc.tensor.matmul(
            pt, lhsT=conf[:, _W2 : _W2 + 32], rhs=rw,
            start=True, stop=True,
        )
        gm = small.tile([GP, 64], F32, tag="gm")
        nc.vector.tensor_tensor(
            out=gm, in0=G, in1=conb[:, _M2C : _M2C + 64], op=mult
        )
        cmc = small.tile([GP, 8], F32, tag="cmc")
        nc.vector.reduce_sum(
            out=cmc,
            in_=gm.rearrange("p (g f) -> p f g", g=8),
            axis=mybir.AxisListType.X,
        )
        # [DVE] fold: args = pt + cmc[x-rows]
        args = small.tile([32, 8], F32, tag="args")
        nc.vector.tensor_tensor(
            out=args, in0=pt, in1=cmc[0:32, :], op=add
        )

        # [ACT] exp over [32,8]: scale -beta/DK, bias carries own norm
        w = small.tile([32, 8], F32, tag="w")
        nc.scalar.activation(
            out=w, in_=args, func=EXP,
            scale=conf[0:32, _BV : _BV + 1], bias=bias,
        )

        # -> DMA out (scalar engine: no cross-engine hop after the ACT)
        nc.scalar.dma_start(out=res_d[:], in_=w)

        # postamble clock probe: keep the PE sequencer hot through the
        # drain window (gated on w so the spins run at the very end)
        if POST_SPIN:
            ptj = psum.tile([8, 8], F32, tag="ptj")
            for i in range(POST_SPIN):
                nc.tensor.matmul(
                    ptj, lhsT=w, rhs=w, start=True, stop=True,
                    skip_group_check=True,
                )

    nc.compile()
    return nc


_PROG = None
_CONSTS = None


def _get_prog():
    global _PROG
    if _PROG is None:
        _PROG = _build_program()
    return _PROG


def _make_in_maps(x, y, t):
    global _CONSTS
    if _CONSTS is None:
        _CONSTS = _build_consts()
    conf0, conb = _CONSTS
    beta = BETA_START + (BETA_END - BETA_START) * (
        t.astype(np.float64) / (NUM_TIMESTEPS - 1)
    )
    in_maps = []
    for c in range(NCORES):
        xc = x[c * NS : (c + 1) * NS].reshape(NS * M, D)
        yc = y[c * NS : (c + 1) * NS].reshape(NS * M, D)
        z = np.concatenate([xc, yc], axis=0)[:, :DK]  # [64, DK]
        # feature-major: zt[p, k*64 + r] = z[r, k*128 + p]
        zt = np.ascontiguousarray(
            z.reshape(R, NCH, 128).transpose(2, 1, 0).reshape(128, FREE)
        ).astype(NP_FP8)
        conf = conf0.copy()
        bcore = np.repeat(beta[c * NS : (c + 1) * NS], M)  # [32]
        conf[0:32, _BV] = (-bcore / DK).astype(np.float32)
        in_maps.append({"zt": zt, "conf": conf, "conb": conb})
    return in_maps


def _fold(res):
    """res [32, 8] per core -> (conf_sum, pair_sum) per sample [4]."""
    w = res.reshape(NS, M, M)                      # [s, j, f]
    conf_sum = np.einsum("sjj->s", w)
    total = w.sum(axis=(1, 2))
    return conf_sum, total - conf_sum


def _run(x, y, t, trace=False, **spmd_kwargs):
    x = np.asarray(x, dtype=np.float32)
    y = np.asarray(y, dtype=np.float32)
    t = np.asarray(t, dtype=np.int32)
    nc = _get_prog()
    in_maps = _make_in_maps(x, y, t)
    br = run_bass_kernel_spmd(
        nc, in_maps, list(range(NCORES)), trace=trace, **spmd_kwargs
    )
    confs, pairs = [], []
    for r in br.results:
        cs, ps = _fold(np.asarray(r["res"], dtype=np.float32))
        confs.append(cs)
        pairs.append(ps)
    conf_sum = np.concatenate(confs)
    pair_sum = np.concatenate(pairs)
    conf = conf_sum / M
    inter = pair_sum / (M * (M - 1))
    im = (LAMBDA_VAL / 2.0) * inter
    score = im - conf
    outs = tuple(
        np.ascontiguousarray(v, dtype=np.float32)
        for v in (score, conf, inter, im)
    )
    return outs, br


def kernel(x, y, t):
    """(score, confinement, interaction, interaction_mult), each [32] f32."""
    outs, _ = _run(x, y, t)
    return outs
